# revision 3
# baseline (speedup 1.0000x reference)
"""Causal self-attention (B=4, S=2048, D=1024, single head, fp32) on 8 trn2
NeuronCores.

Sharding: core 2*b + c handles batch b with the parity-c half of the keys
(global key rows 2*i + c), over ALL queries — a flash-attention split over
the key dimension. Each core returns unnormalized softmax numerators
o = sum_k exp(s~ - m~) v plus per-row stats (m = raw-score row max,
l = sum exp); the host combines the two key-halves exactly.

SPMD trick: one program serves both parities. The host pair-swaps the rows
of x for odd cores (rows [1,0,3,2,...]), so each core's keys sit at even
row positions and the on-chip stride-2 access pattern is parity-free. The
causal boundary masks (which depend on the parity) ship as a small
per-core input; the host pair-swaps the outputs of odd cores back.

Schedule: x is processed in 4 query-chunks of 512 rows. Per chunk c the
kernel (a) DMA-block-permutes + DVE-stream-transposes the NEXT chunk into
xT while (b) the PE projects chunk c: kT = Wk^T x_k, v = x_k Wv, and
G = Wq^T k^T (the q-projection is never materialized: scores = x (Wq k^T)
by associativity, which halves the q cost since keys < queries per core),
then (c) runs attention for query blocks j = 4c..4c+3 with a 1-deep
softmax pipeline (scores_j on PE, then attn@v of j-1 while softmax_j runs
on DVE/Scalar). Scores stay in PSUM: the causal mask is added in place by
the DVE, row-max is reduced from PSUM, and the Scalar engine's Exp reads
PSUM directly (accumulating l on the fly). PSUM evictions and fp32->fp32r
rounding copies run on the Scalar engine; weight loads are casting DMAs
issued by GPSIMD, keeping the DVE free for stream transposes.
"""
import math
import numpy as np

import concourse.bacc as bacc
import concourse.mybir as mybir
from concourse import tile
from concourse.masks import make_identity
from concourse.bass_utils import run_bass_kernel_spmd

B, S, D = 4, 2048, 1024
P = 128
DT = D // P          # 8 d-tiles (contraction)
ET = D // P          # 8 e-tiles (output feature)
NQB = S // P         # 16 query blocks
HKT = 8              # compacted key blocks per core (1024 keys)
NCH = 4              # query chunks of 512 rows
CQ = S // NCH        # 512 queries per chunk
CK = CQ // 2         # 256 compacted keys per chunk
INV_SQRT_D = 1.0 / math.sqrt(D)
NEG = -1e30

F32 = mybir.dt.float32
F32R = mybir.dt.float32r
BF16 = mybir.dt.bfloat16


def _ceil_div(a, b):
    return (a + b - 1) // b


def build_nc():
    nc = bacc.Bacc("TRN2", target_bir_lowering=False)
    x_p = nc.declare_dram_parameter("x", [S, D], F32, isOutput=False)
    wq_p = nc.declare_dram_parameter("wq", [D, D], F32, isOutput=False)
    wk_p = nc.declare_dram_parameter("wk", [D, D], F32, isOutput=False)
    wv_p = nc.declare_dram_parameter("wv", [D, D], F32, isOutput=False)
    mask_p = nc.declare_dram_parameter("mask", [P, 2, P], F32, isOutput=False)
    o_p = nc.declare_dram_parameter("o", [S, D], F32, isOutput=True)
    m_p = nc.declare_dram_parameter("m", [P, NQB], F32, isOutput=True)
    l_p = nc.declare_dram_parameter("l", [P, NQB], F32, isOutput=True)

    with tile.TileContext(nc) as tc:
        with (
            tc.tile_pool(name="const_pool", bufs=1) as const_pool,
            tc.tile_pool(name="w_pool", bufs=1) as w_pool,
            tc.tile_pool(name="g_pool", bufs=1) as g_pool,
            tc.tile_pool(name="v_pool", bufs=1) as v_pool,
            tc.tile_pool(name="xT_pool", bufs=2) as xT_pool,
            tc.tile_pool(name="y_pool", bufs=2) as y_pool,
            tc.tile_pool(name="z_pool", bufs=1) as z_pool,
            tc.tile_pool(name="kt_pool", bufs=1) as kt_pool,
            tc.tile_pool(name="at_pool", bufs=2) as at_pool,
            tc.tile_pool(name="atT_pool", bufs=1) as atT_pool,
            tc.tile_pool(name="ob_pool", bufs=1) as ob_pool,
            tc.tile_pool(name="st_pool", bufs=2) as st_pool,
            tc.tile_pool(name="psP_pool", bufs=1, space="PSUM") as psP_pool,
            tc.tile_pool(name="psT_pool", bufs=2, space="PSUM") as psT_pool,
            tc.tile_pool(name="psO_pool", bufs=1, space="PSUM") as psO_pool,
        ):
            ident_bf = const_pool.tile([P, P], BF16)
            mask_sb = const_pool.tile([P, 2, P], F32)
            m_all = const_pool.tile([P, NQB], F32)
            l_all = const_pool.tile([P, NQB], F32)
            make_identity(nc, ident_bf[:])
            nc.sync.dma_start(out=mask_sb[:], in_=mask_p[:])

            wk_sb = w_pool.tile([P, DT, D], F32R)   # [d_p, dt, e]
            wv_sb = w_pool.tile([P, DT, D], F32R)   # [d_p, dt, e]
            wqT = w_pool.tile([P, ET, D], F32R)     # [e_p, et, d]
            G = g_pool.tile([P, DT, HKT * P], F32R)  # [d_p, dt, k]
            vv = v_pool.tile([P, HKT, D], BF16)      # [k_p, kb, e]

            # weight loads: casting DMAs (f32 -> f32r) issued by gpsimd
            for dt in range(DT):
                nc.gpsimd.dma_start(
                    out=wk_sb[:, dt, :],
                    in_=wk_p[dt * P:(dt + 1) * P, :])
                nc.gpsimd.dma_start(
                    out=wv_sb[:, dt, :],
                    in_=wv_p[dt * P:(dt + 1) * P, :])

            def transpose_block(src_ap, dst, dst_col, name):
                """One [128, D] DRAM block -> dst[:, :, dst_col:dst_col+128]
                transposed ([d_p, dt, row]). DMA applies the 32x32 block grid
                permute, DVE stream-transpose finishes each block, Scalar
                rounds f32 -> f32r into dst."""
                y_t = y_pool.tile([P, DT, P], F32, tag="y", name=f"y_{name}")
                z_t = z_pool.tile([P, DT, P], F32, tag="z", name=f"z_{name}")
                x_r = src_ap.rearrange(
                    "(b w) (dt a u) -> a w dt b u",
                    b=4, w=32, dt=DT, a=4, u=32)
                for a in range(4):
                    nc.sync.dma_start(
                        out=y_t[32 * a:32 * (a + 1), :, :].rearrange(
                            "w dt (b u) -> w dt b u", b=4),
                        in_=x_r[a])
                for dt in range(DT):
                    nc.vector.transpose(z_t[:, dt, :], y_t[:, dt, :])
                nc.scalar.copy(dst[:, :, dst_col:dst_col + P], z_t[:])

            xts = {}

            def prefetch_chunk(c):
                xts[c] = xT_pool.tile([P, DT, CQ], F32R, tag="xt",
                                      name=f"xt{c}")
                for k in range(4):
                    st = 4 * c + k
                    transpose_block(x_p[st * P:(st + 1) * P, :], xts[c],
                                    k * P, f"x{st}")

            # prologue: x chunk 0, then Wq^T via the same machinery
            prefetch_chunk(0)
            for db in range(DT):
                transpose_block(wq_p[db * P:(db + 1) * P, :], wqT,
                                db * P, f"wq{db}")

            state = {}

            def issue_scores(j):
                nkb = j // 2 + 1
                ncols = nkb * P
                nch = _ceil_div(ncols, 512)
                c = j // 4
                qoff = (j % 4) * P
                xt = xts[c]
                ps_list = []
                for ch in range(nch):
                    ncc = min(512, ncols - ch * 512)
                    ps = psP_pool.tile([P, 512], F32,
                                       tag=f"b{2 * (j % 2) + ch}",
                                       name=f"psS{j}_{ch}")
                    for db in range(DT):
                        nc.tensor.matmul(
                            ps[:, :ncc],
                            xt[:, db, qoff:qoff + P],
                            G[:, db, ch * 512:ch * 512 + ncc],
                            start=(db == 0), stop=(db == DT - 1))
                    ps_list.append((ps, ncc))
                # causal mask on the boundary block, in place in PSUM
                bl_ch = (ncols - P) // 512
                off = (ncols - P) % 512
                nc.vector.tensor_add(
                    ps_list[bl_ch][0][:, off:off + P],
                    ps_list[bl_ch][0][:, off:off + P],
                    mask_sb[:, j % 2, :])
                # row max (raw scores)
                if nch == 1:
                    nc.vector.reduce_max(m_all[:, j:j + 1],
                                         ps_list[0][0][:, :ps_list[0][1]],
                                         axis=mybir.AxisListType.X)
                else:
                    mp = st_pool.tile([P, 2], F32, tag="mp", name=f"mp{j}")
                    for ch, (ps, ncc) in enumerate(ps_list):
                        nc.vector.reduce_max(mp[:, ch:ch + 1], ps[:, :ncc],
                                             axis=mybir.AxisListType.X)
                    nc.vector.reduce_max(m_all[:, j:j + 1], mp[:],
                                         axis=mybir.AxisListType.X)
                neg = st_pool.tile([P, 1], F32, tag="neg", name=f"neg{j}")
                nc.vector.tensor_scalar_mul(neg[:], m_all[:, j:j + 1],
                                            -INV_SQRT_D)
                at = at_pool.tile([P, NQB * 64], BF16, tag="at",
                                  name=f"at{j}")
                if nch == 1:
                    nc.scalar.activation(
                        at[:, :ncols], ps_list[0][0][:, :ncols],
                        mybir.ActivationFunctionType.Exp,
                        bias=neg[:], scale=INV_SQRT_D,
                        accum_out=l_all[:, j:j + 1])
                else:
                    lp = st_pool.tile([P, 2], F32, tag="lp", name=f"lp{j}")
                    for ch, (ps, ncc) in enumerate(ps_list):
                        nc.scalar.activation(
                            at[:, ch * 512:ch * 512 + ncc], ps[:, :ncc],
                            mybir.ActivationFunctionType.Exp,
                            bias=neg[:], scale=INV_SQRT_D,
                            accum_out=lp[:, ch:ch + 1])
                    nc.gpsimd.tensor_add(l_all[:, j:j + 1],
                                         lp[:, 0:1], lp[:, 1:2])
                state[j] = (at, nkb)

            def issue_fin(j):
                at, nkb = state.pop(j)
                atT = atT_pool.tile([P, HKT, P], BF16, tag="atT",
                                    name=f"atT{j}")
                done = 0
                gi = 0
                while done < nkb:
                    nn = min(4, nkb - done)
                    psT = psT_pool.tile([P, 512], BF16, tag="pt",
                                        name=f"psT{j}_{gi}")
                    for i in range(nn):
                        nc.tensor.transpose(
                            psT[:, i * P:(i + 1) * P],
                            at[:, (done + i) * P:(done + i + 1) * P],
                            ident_bf[:])
                    nc.vector.tensor_copy(
                        atT[:, done:done + nn, :],
                        psT[:, :nn * P].rearrange("p (a b) -> p a b", a=nn))
                    done += nn
                    gi += 1
                psO = [psO_pool.tile([P, 512], F32, tag=f"o{eb}",
                                     name=f"psO{j}_{eb}")
                       for eb in range(2)]
                for kb in range(nkb):
                    for eb in range(2):
                        nc.tensor.matmul(
                            psO[eb][:],
                            atT[:, kb, :],
                            vv[:, kb, eb * 512:(eb + 1) * 512],
                            start=(kb == 0), stop=(kb == nkb - 1))
                for eb in range(2):
                    o_sb = ob_pool.tile([P, 512], F32, tag=f"os{eb}",
                                        name=f"o{j}_{eb}")
                    if eb == 0:
                        nc.vector.tensor_copy(o_sb[:], psO[eb][:])
                    else:
                        nc.scalar.copy(o_sb[:], psO[eb][:])
                    nc.sync.dma_start(
                        out=o_p[j * P:(j + 1) * P, eb * 512:(eb + 1) * 512],
                        in_=o_sb[:])

            pending = None
            for c in range(NCH):
                if c + 1 < NCH:
                    prefetch_chunk(c + 1)
                xt = xts[c]
                # even-position (this core's keys) stride-2 view
                xk = xt.rearrange("p d (s two) -> p d two s", two=2)

                # kT[e, k] for this chunk's 256 compacted keys
                kt = kt_pool.tile([P, ET, CK], F32R, tag="kt", name=f"kt{c}")
                for pr in range(4):
                    ps = psP_pool.tile([P, 512], F32, tag=f"b{pr}",
                                       name=f"psK{c}_{pr}")
                    for half in range(2):
                        et = 2 * pr + half
                        for dt in range(DT):
                            nc.tensor.matmul(
                                ps[:, half * CK:(half + 1) * CK],
                                wk_sb[:, dt, et * P:(et + 1) * P],
                                xk[:, dt, 0, :],
                                start=(dt == 0), stop=(dt == DT - 1))
                    nc.scalar.copy(
                        kt[:, 2 * pr:2 * pr + 2, :],
                        ps[:].rearrange("p (a b) -> p a b", a=2))

                # v[k, e] for this chunk's 2 key blocks
                for kb in range(2):
                    for eb in range(2):
                        ps = psP_pool.tile([P, 512], F32,
                                           tag=f"b{2 * kb + eb}",
                                           name=f"psV{c}_{kb}_{eb}")
                        for dt in range(DT):
                            nc.tensor.matmul(
                                ps[:],
                                xk[:, dt, 0, kb * P:(kb + 1) * P],
                                wv_sb[:, dt, eb * 512:(eb + 1) * 512],
                                start=(dt == 0), stop=(dt == DT - 1))
                        nc.scalar.copy(vv[:, 2 * c + kb,
                                          eb * 512:(eb + 1) * 512], ps[:])

                # G[d, k] = sum_e WqT[e, d] kT[e, k] for this chunk's keys
                for pr in range(4):
                    ps = psP_pool.tile([P, 512], F32, tag=f"b{pr}",
                                       name=f"psG{c}_{pr}")
                    for half in range(2):
                        db = 2 * pr + half
                        for et in range(ET):
                            nc.tensor.matmul(
                                ps[:, half * CK:(half + 1) * CK],
                                wqT[:, et, db * P:(db + 1) * P],
                                kt[:, et, :],
                                start=(et == 0), stop=(et == ET - 1))
                    nc.scalar.copy(
                        G[:, 2 * pr:2 * pr + 2, c * CK:(c + 1) * CK],
                        ps[:].rearrange("p (a b) -> p a b", a=2))

                for j in range(4 * c, 4 * c + 4):
                    issue_scores(j)
                    if pending is not None:
                        issue_fin(pending)
                    pending = j
            issue_fin(pending)
            nc.sync.dma_start(out=m_p[:], in_=m_all[:])
            nc.sync.dma_start(out=l_p[:], in_=l_all[:])
    nc.finalize()
    return nc


def _boundary_masks(c):
    """mask[row, par, i]: 0 if compacted key i is causally valid for local
    query row `row` of an even (par=0) / odd (par=1) query block, else -1e30.

    For parity-1 cores, x rows arrive pair-swapped, so the query at local
    position `row` is global row 128*j + r_local with
    r_local = row+1 (even row) / row-1 (odd row). Key i is global row
    256*(j//2) + 2*i + c. Valid iff 2*i + c <= par*128 + r_local.
    """
    mask = np.full((P, 2, P), NEG, dtype=np.float32)
    for row in range(P):
        r_local = row if c == 0 else (row + 1 if row % 2 == 0 else row - 1)
        for par in range(2):
            lim = (par * P + r_local - c) // 2
            if lim >= 0:
                mask[row, par, :min(lim + 1, P)] = 0.0
    return mask


_PAIRSWAP = np.arange(S).reshape(-1, 2)[:, ::-1].reshape(-1)
_CACHED_NC = None


def _make_in_maps(x, Wq, Wk, Wv):
    x = np.asarray(x, dtype=np.float32)
    Wq = np.ascontiguousarray(np.asarray(Wq, dtype=np.float32))
    Wk = np.ascontiguousarray(np.asarray(Wk, dtype=np.float32))
    Wv = np.ascontiguousarray(np.asarray(Wv, dtype=np.float32))
    masks = [_boundary_masks(0), _boundary_masks(1)]
    in_maps = []
    for core in range(8):
        b, c = core // 2, core % 2
        xb = x[b] if c == 0 else x[b][_PAIRSWAP]
        in_maps.append({
            "x": np.ascontiguousarray(xb),
            "wq": Wq, "wk": Wk, "wv": Wv,
            "mask": masks[c],
        })
    return in_maps


def _combine(res):
    out = np.empty((B, S, D), dtype=np.float32)
    for b in range(B):
        r0, r1 = res.results[2 * b], res.results[2 * b + 1]
        o0 = r0["o"]
        # parity-1 core computed on pair-swapped query rows; swap back
        def stat(r, key):
            return np.ascontiguousarray(r[key].T).reshape(S, 1)
        m0, l0 = stat(r0, "m"), stat(r0, "l")
        o1 = r1["o"][_PAIRSWAP]
        m1 = stat(r1, "m")[_PAIRSWAP]
        l1 = stat(r1, "l")[_PAIRSWAP]
        ms0 = m0.astype(np.float64) * INV_SQRT_D
        ms1 = m1.astype(np.float64) * INV_SQRT_D
        mm = np.maximum(ms0, ms1)
        w0 = np.exp(ms0 - mm)
        w1 = np.exp(ms1 - mm)
        num = w0 * o0.astype(np.float64) + w1 * o1.astype(np.float64)
        den = w0 * l0.astype(np.float64) + w1 * l1.astype(np.float64)
        out[b] = (num / den).astype(np.float32)
    return out


def kernel(x, Wq, Wk, Wv):
    global _CACHED_NC
    if _CACHED_NC is None:
        _CACHED_NC = build_nc()
    in_maps = _make_in_maps(x, Wq, Wk, Wv)
    res = run_bass_kernel_spmd(_CACHED_NC, in_maps, list(range(8)))
    return _combine(res)


# revision 4
# speedup vs baseline: 1.3639x; 1.3639x over previous
"""Causal self-attention (B=4, S=2048, D=1024, single head, fp32) on 8 trn2
NeuronCores.

Sharding: core 2*b + c handles batch b with the parity-c half of the keys
(global key rows 2*i + c), over ALL queries — a flash-attention split over
the key dimension. Each core returns unnormalized softmax numerators
o = sum_k exp(s~ - m~) v plus per-row stats (m = raw-score row max,
l = sum exp); the host combines the two key-halves exactly.

SPMD trick: one program serves both parities. The host pair-swaps the rows
of x for odd cores (rows [1,0,3,2,...]), so each core's keys sit at even
row positions and the on-chip stride-2 access pattern is parity-free. The
causal boundary masks (which depend on the parity) ship as a small
per-core input; the host pair-swaps the outputs of odd cores back.

Schedule: x is processed in 4 query-chunks of 512 rows. Per chunk c the
kernel (a) DMA-block-permutes + DVE-stream-transposes the NEXT chunk into
xT while (b) the PE projects chunk c: kT = Wk^T x_k, v = x_k Wv, and
G = Wq^T k^T (the q-projection is never materialized: scores = x (Wq k^T)
by associativity, which halves the q cost since keys < queries per core),
then (c) runs attention for query blocks j = 4c..4c+3 with a 1-deep
softmax pipeline (scores_j on PE, then attn@v of j-1 while softmax_j runs
on DVE/Scalar). Scores stay in PSUM: the causal mask is added in place by
the DVE, row-max is reduced from PSUM, and the Scalar engine's Exp reads
PSUM directly (accumulating l on the fly).

All matmul operands are bf16 (PSUM accumulation stays fp32): bf16 weight
loads hide fully under the matmuls, the f32->bf16 conversion rides the
DMA for free (GPSIMD-initiated casting DMAs), and the DVE stream
transpose writes matmul-ready bf16 directly. Wq^T is produced by PE
transposes during the prologue (warming the PE while x streams in).
PSUM evictions run on the Scalar engine.
"""
import math
import numpy as np

import concourse.bacc as bacc
import concourse.mybir as mybir
from concourse import tile
from concourse.masks import make_identity
from concourse.bass_utils import run_bass_kernel_spmd

B, S, D = 4, 2048, 1024
P = 128
DT = D // P          # 8 d-tiles (contraction)
ET = D // P          # 8 e-tiles (output feature)
NQB = S // P         # 16 query blocks
HKT = 8              # compacted key blocks per core (1024 keys)
NCH = 4              # query chunks of 512 rows
CQ = S // NCH        # 512 queries per chunk
CK = CQ // 2         # 256 compacted keys per chunk
INV_SQRT_D = 1.0 / math.sqrt(D)
NEG = -1e30

F32 = mybir.dt.float32
BF16 = mybir.dt.bfloat16


def _ceil_div(a, b):
    return (a + b - 1) // b


def build_nc():
    nc = bacc.Bacc("TRN2", target_bir_lowering=False)
    x_p = nc.declare_dram_parameter("x", [S, D], F32, isOutput=False)
    wq_p = nc.declare_dram_parameter("wq", [D, D], F32, isOutput=False)
    wk_p = nc.declare_dram_parameter("wk", [D, D], F32, isOutput=False)
    wv_p = nc.declare_dram_parameter("wv", [D, D], F32, isOutput=False)
    mask_p = nc.declare_dram_parameter("mask", [P, 2, P], F32, isOutput=False)
    o_p = nc.declare_dram_parameter("o", [S, D], F32, isOutput=True)
    m_p = nc.declare_dram_parameter("m", [P, NQB], F32, isOutput=True)
    l_p = nc.declare_dram_parameter("l", [P, NQB], F32, isOutput=True)

    with tile.TileContext(nc) as tc:
        with (
            tc.tile_pool(name="const_pool", bufs=1) as const_pool,
            tc.tile_pool(name="w_pool", bufs=1) as w_pool,
            tc.tile_pool(name="g_pool", bufs=1) as g_pool,
            tc.tile_pool(name="v_pool", bufs=1) as v_pool,
            tc.tile_pool(name="xT_pool", bufs=2) as xT_pool,
            tc.tile_pool(name="y_pool", bufs=4) as y_pool,
            tc.tile_pool(name="wqs_pool", bufs=2) as wqs_pool,
            tc.tile_pool(name="kt_pool", bufs=2) as kt_pool,
            tc.tile_pool(name="at_pool", bufs=2) as at_pool,
            tc.tile_pool(name="atT_pool", bufs=2) as atT_pool,
            tc.tile_pool(name="ob_pool", bufs=2) as ob_pool,
            tc.tile_pool(name="st_pool", bufs=2) as st_pool,
            tc.tile_pool(name="psP_pool", bufs=1, space="PSUM") as psP_pool,
            tc.tile_pool(name="psT_pool", bufs=2, space="PSUM") as psT_pool,
            tc.tile_pool(name="psO_pool", bufs=1, space="PSUM") as psO_pool,
        ):
            ident_bf = const_pool.tile([P, P], BF16)
            mask_sb = const_pool.tile([P, 2, P], F32)
            m_all = const_pool.tile([P, NQB], F32)
            l_all = const_pool.tile([P, NQB], F32)
            make_identity(nc, ident_bf[:])
            nc.sync.dma_start(out=mask_sb[:], in_=mask_p[:])

            wk_sb = w_pool.tile([P, DT, D], BF16)   # [d_p, dt, e]
            wv_sb = w_pool.tile([P, DT, D], BF16)   # [d_p, dt, e]
            wqT = w_pool.tile([P, ET, D], BF16)     # [e_p, et, d]
            G = g_pool.tile([P, DT, HKT * P], BF16)  # [d_p, dt, k]
            vv = v_pool.tile([P, HKT, D], BF16)      # [k_p, kb, e]

            # weight loads: casting DMAs (f32 -> bf16) issued by gpsimd
            for dt in range(DT):
                nc.gpsimd.dma_start(
                    out=wk_sb[:, dt, :],
                    in_=wk_p[dt * P:(dt + 1) * P, :])
                nc.gpsimd.dma_start(
                    out=wv_sb[:, dt, :],
                    in_=wv_p[dt * P:(dt + 1) * P, :])

            def transpose_block(src_ap, dst, dst_col, name):
                """One [128, D] DRAM block -> dst[:, :, dst_col:dst_col+128]
                transposed ([d_p, dt, row]). The casting DMA applies the
                32x32 block grid permute and rounds f32 -> bf16; the DVE
                stream transpose writes matmul-ready bf16 into dst."""
                y_t = y_pool.tile([P, DT, P], BF16, tag="y", name=f"y_{name}")
                x_r = src_ap.rearrange(
                    "(b w) (dt a u) -> a w dt b u",
                    b=4, w=32, dt=DT, a=4, u=32)
                for a in range(4):
                    nc.gpsimd.dma_start(
                        out=y_t[32 * a:32 * (a + 1), :, :].rearrange(
                            "w dt (b u) -> w dt b u", b=4),
                        in_=x_r[a])
                for dt in range(DT):
                    nc.vector.transpose(dst[:, dt, dst_col:dst_col + P],
                                        y_t[:, dt, :])

            xts = {}

            def prefetch_chunk(c):
                xts[c] = xT_pool.tile([P, DT, CQ], BF16, tag="xt",
                                      name=f"xt{c}")
                for k in range(4):
                    st = 4 * c + k
                    transpose_block(x_p[st * P:(st + 1) * P, :], xts[c],
                                    k * P, f"x{st}")

            # prologue: Wq^T via PE transposes (PE warms up immediately);
            # x chunk 0 streams through DMA+DVE concurrently.
            wq_stage = []
            for db in range(DT):
                wqs = wqs_pool.tile([P, D], BF16, tag="wqs", name=f"wqs{db}")
                nc.gpsimd.dma_start(
                    out=wqs[:], in_=wq_p[db * P:(db + 1) * P, :])
                wq_stage.append(wqs)
                if db == 0:
                    prefetch_chunk(0)
            for db in range(DT):
                wqs = wq_stage[db]
                for g in range(2):
                    psT = psT_pool.tile([P, 512], BF16, tag="pt",
                                        name=f"psW{db}_{g}")
                    for i in range(4):
                        et = 4 * g + i
                        nc.tensor.transpose(
                            psT[:, i * P:(i + 1) * P],
                            wqs[:, et * P:(et + 1) * P],
                            ident_bf[:])
                    nc.scalar.copy(
                        wqT[:, 4 * g:4 * g + 4, db * P:(db + 1) * P],
                        psT[:].rearrange("p (a b) -> p a b", a=4))

            state = {}

            def issue_scores(j):
                nkb = j // 2 + 1
                ncols = nkb * P
                nch = _ceil_div(ncols, 512)
                c = j // 4
                qoff = (j % 4) * P
                xt = xts[c]
                ps_list = []
                for ch in range(nch):
                    ncc = min(512, ncols - ch * 512)
                    ps = psP_pool.tile([P, 512], F32,
                                       tag=f"b{2 * (j % 2) + ch}",
                                       name=f"psS{j}_{ch}")
                    for db in range(DT):
                        nc.tensor.matmul(
                            ps[:, :ncc],
                            xt[:, db, qoff:qoff + P],
                            G[:, db, ch * 512:ch * 512 + ncc],
                            start=(db == 0), stop=(db == DT - 1))
                    ps_list.append((ps, ncc))
                # causal mask on the boundary block, in place in PSUM
                bl_ch = (ncols - P) // 512
                off = (ncols - P) % 512
                nc.vector.tensor_add(
                    ps_list[bl_ch][0][:, off:off + P],
                    ps_list[bl_ch][0][:, off:off + P],
                    mask_sb[:, j % 2, :])
                # row max (raw scores)
                if nch == 1:
                    nc.vector.reduce_max(m_all[:, j:j + 1],
                                         ps_list[0][0][:, :ps_list[0][1]],
                                         axis=mybir.AxisListType.X)
                else:
                    mp = st_pool.tile([P, 2], F32, tag="mp", name=f"mp{j}")
                    for ch, (ps, ncc) in enumerate(ps_list):
                        nc.vector.reduce_max(mp[:, ch:ch + 1], ps[:, :ncc],
                                             axis=mybir.AxisListType.X)
                    nc.vector.reduce_max(m_all[:, j:j + 1], mp[:],
                                         axis=mybir.AxisListType.X)
                neg = st_pool.tile([P, 1], F32, tag="neg", name=f"neg{j}")
                nc.vector.tensor_scalar_mul(neg[:], m_all[:, j:j + 1],
                                            -INV_SQRT_D)
                at = at_pool.tile([P, NQB * 64], BF16, tag="at",
                                  name=f"at{j}")
                if nch == 1:
                    nc.scalar.activation(
                        at[:, :ncols], ps_list[0][0][:, :ncols],
                        mybir.ActivationFunctionType.Exp,
                        bias=neg[:], scale=INV_SQRT_D,
                        accum_out=l_all[:, j:j + 1])
                else:
                    lp = st_pool.tile([P, 2], F32, tag="lp", name=f"lp{j}")
                    for ch, (ps, ncc) in enumerate(ps_list):
                        nc.scalar.activation(
                            at[:, ch * 512:ch * 512 + ncc], ps[:, :ncc],
                            mybir.ActivationFunctionType.Exp,
                            bias=neg[:], scale=INV_SQRT_D,
                            accum_out=lp[:, ch:ch + 1])
                    nc.gpsimd.tensor_add(l_all[:, j:j + 1],
                                         lp[:, 0:1], lp[:, 1:2])
                state[j] = (at, nkb)

            def issue_fin(j):
                at, nkb = state.pop(j)
                atT = atT_pool.tile([P, HKT, P], BF16, tag="atT",
                                    name=f"atT{j}")
                done = 0
                gi = 0
                while done < nkb:
                    nn = min(4, nkb - done)
                    psT = psT_pool.tile([P, 512], BF16, tag="pt",
                                        name=f"psT{j}_{gi}")
                    for i in range(nn):
                        nc.tensor.transpose(
                            psT[:, i * P:(i + 1) * P],
                            at[:, (done + i) * P:(done + i + 1) * P],
                            ident_bf[:])
                    nc.vector.tensor_copy(
                        atT[:, done:done + nn, :],
                        psT[:, :nn * P].rearrange("p (a b) -> p a b", a=nn))
                    done += nn
                    gi += 1
                psO = [psO_pool.tile([P, 512], F32, tag=f"o{eb}",
                                     name=f"psO{j}_{eb}")
                       for eb in range(2)]
                for kb in range(nkb):
                    for eb in range(2):
                        nc.tensor.matmul(
                            psO[eb][:],
                            atT[:, kb, :],
                            vv[:, kb, eb * 512:(eb + 1) * 512],
                            start=(kb == 0), stop=(kb == nkb - 1))
                for eb in range(2):
                    o_sb = ob_pool.tile([P, 512], F32, tag=f"os{eb}",
                                        name=f"o{j}_{eb}")
                    if eb == 0:
                        nc.vector.tensor_copy(o_sb[:], psO[eb][:])
                    else:
                        nc.scalar.copy(o_sb[:], psO[eb][:])
                    nc.sync.dma_start(
                        out=o_p[j * P:(j + 1) * P, eb * 512:(eb + 1) * 512],
                        in_=o_sb[:])

            pending = None
            for c in range(NCH):
                if c + 1 < NCH:
                    prefetch_chunk(c + 1)
                xt = xts[c]
                # even-position (this core's keys) stride-2 view
                xk = xt.rearrange("p d (s two) -> p d two s", two=2)

                # kT[e, k] for this chunk's 256 compacted keys
                kt = kt_pool.tile([P, ET, CK], BF16, tag="kt", name=f"kt{c}")
                for pr in range(4):
                    ps = psP_pool.tile([P, 512], F32, tag=f"b{pr}",
                                       name=f"psK{c}_{pr}")
                    for half in range(2):
                        et = 2 * pr + half
                        for dt in range(DT):
                            nc.tensor.matmul(
                                ps[:, half * CK:(half + 1) * CK],
                                wk_sb[:, dt, et * P:(et + 1) * P],
                                xk[:, dt, 0, :],
                                start=(dt == 0), stop=(dt == DT - 1))
                    nc.scalar.copy(
                        kt[:, 2 * pr:2 * pr + 2, :],
                        ps[:].rearrange("p (a b) -> p a b", a=2))

                # v[k, e] for this chunk's 2 key blocks
                for kb in range(2):
                    for eb in range(2):
                        ps = psP_pool.tile([P, 512], F32,
                                           tag=f"b{2 * kb + eb}",
                                           name=f"psV{c}_{kb}_{eb}")
                        for dt in range(DT):
                            nc.tensor.matmul(
                                ps[:],
                                xk[:, dt, 0, kb * P:(kb + 1) * P],
                                wv_sb[:, dt, eb * 512:(eb + 1) * 512],
                                start=(dt == 0), stop=(dt == DT - 1))
                        nc.scalar.copy(vv[:, 2 * c + kb,
                                          eb * 512:(eb + 1) * 512], ps[:])

                # G[d, k] = sum_e WqT[e, d] kT[e, k] for this chunk's keys
                for pr in range(4):
                    ps = psP_pool.tile([P, 512], F32, tag=f"b{pr}",
                                       name=f"psG{c}_{pr}")
                    for half in range(2):
                        db = 2 * pr + half
                        for et in range(ET):
                            nc.tensor.matmul(
                                ps[:, half * CK:(half + 1) * CK],
                                wqT[:, et, db * P:(db + 1) * P],
                                kt[:, et, :],
                                start=(et == 0), stop=(et == ET - 1))
                    nc.scalar.copy(
                        G[:, 2 * pr:2 * pr + 2, c * CK:(c + 1) * CK],
                        ps[:].rearrange("p (a b) -> p a b", a=2))

                for j in range(4 * c, 4 * c + 4):
                    issue_scores(j)
                    if pending is not None:
                        issue_fin(pending)
                    pending = j
            issue_fin(pending)
            nc.sync.dma_start(out=m_p[:], in_=m_all[:])
            nc.sync.dma_start(out=l_p[:], in_=l_all[:])
    nc.finalize()
    return nc


def _boundary_masks(c):
    """mask[row, par, i]: 0 if compacted key i is causally valid for local
    query row `row` of an even (par=0) / odd (par=1) query block, else -1e30.

    For parity-1 cores, x rows arrive pair-swapped, so the query at local
    position `row` is global row 128*j + r_local with
    r_local = row+1 (even row) / row-1 (odd row). Key i is global row
    256*(j//2) + 2*i + c. Valid iff 2*i + c <= par*128 + r_local.
    """
    mask = np.full((P, 2, P), NEG, dtype=np.float32)
    for row in range(P):
        r_local = row if c == 0 else (row + 1 if row % 2 == 0 else row - 1)
        for par in range(2):
            lim = (par * P + r_local - c) // 2
            if lim >= 0:
                mask[row, par, :min(lim + 1, P)] = 0.0
    return mask


_PAIRSWAP = np.arange(S).reshape(-1, 2)[:, ::-1].reshape(-1)
_CACHED_NC = None


def _make_in_maps(x, Wq, Wk, Wv):
    x = np.asarray(x, dtype=np.float32)
    Wq = np.ascontiguousarray(np.asarray(Wq, dtype=np.float32))
    Wk = np.ascontiguousarray(np.asarray(Wk, dtype=np.float32))
    Wv = np.ascontiguousarray(np.asarray(Wv, dtype=np.float32))
    masks = [_boundary_masks(0), _boundary_masks(1)]
    in_maps = []
    for core in range(8):
        b, c = core // 2, core % 2
        xb = x[b] if c == 0 else x[b][_PAIRSWAP]
        in_maps.append({
            "x": np.ascontiguousarray(xb),
            "wq": Wq, "wk": Wk, "wv": Wv,
            "mask": masks[c],
        })
    return in_maps


def _combine(res):
    out = np.empty((B, S, D), dtype=np.float32)
    for b in range(B):
        r0, r1 = res.results[2 * b], res.results[2 * b + 1]
        o0 = r0["o"]
        # parity-1 core computed on pair-swapped query rows; swap back
        def stat(r, key):
            return np.ascontiguousarray(r[key].T).reshape(S, 1)
        m0, l0 = stat(r0, "m"), stat(r0, "l")
        o1 = r1["o"][_PAIRSWAP]
        m1 = stat(r1, "m")[_PAIRSWAP]
        l1 = stat(r1, "l")[_PAIRSWAP]
        ms0 = m0.astype(np.float64) * INV_SQRT_D
        ms1 = m1.astype(np.float64) * INV_SQRT_D
        mm = np.maximum(ms0, ms1)
        w0 = np.exp(ms0 - mm)
        w1 = np.exp(ms1 - mm)
        num = w0 * o0.astype(np.float64) + w1 * o1.astype(np.float64)
        den = w0 * l0.astype(np.float64) + w1 * l1.astype(np.float64)
        out[b] = (num / den).astype(np.float32)
    return out


def kernel(x, Wq, Wk, Wv):
    global _CACHED_NC
    if _CACHED_NC is None:
        _CACHED_NC = build_nc()
    in_maps = _make_in_maps(x, Wq, Wk, Wv)
    res = run_bass_kernel_spmd(_CACHED_NC, in_maps, list(range(8)))
    return _combine(res)
